# revision 6
# baseline (speedup 1.0000x reference)
"""Trainium2 Bass kernel for nn_AttentionLayer (label-attention pooling).

Reference computation:
    weights = tanh(x @ W1.T)                    # [B,S,A]   (A = D = 512)
    att     = softmax(weights @ W2.T, axis=S)   # [B,S,L]
    att_w   = att.swapaxes(1,2)                 # [B,L,S]
    out     = att_w.sum(L) @ x / R  ==  (colsum_l att_w) @ x / R   # [B,D]
    returns (out, att_w)

Sharding: 8 cores = 4 batches x 2 label-halves. L=8921 padded to 9216;
each core owns one batch and 4608 labels (36 tiles of 128).

Device program per core (SPMD, per-core data differs):
  phase 0: wT[a,s] = tanh(W1 @ xT)  resident in SBUF   (f32r matmuls)
  phase 1: per l-tile: z = W2T.T @ wT -> exp (ScalarE, fused row-sum)
           -> reciprocal+mask -> in-place scale (VectorE) -> DMA out;
           colsum pieces via selector matmuls accumulating in 2 PSUM banks.
Host: reassemble att, out = colsum @ x / R (8.4 MFLOP).
"""

import numpy as np

import concourse.bacc as bacc
import concourse.tile as tile
import concourse.mybir as mybir
from concourse.bass_utils import run_bass_kernel_spmd

f32 = mybir.dt.float32
f32r = mybir.dt.float32r

B, S, D = 4, 4096, 512
L = 8921
R = 8921.0
NCORES = 8
LH = 4608           # labels per core (L padded to 2*LH*4... 8*1152? no: 2 halves * 4 batches)
NLT = LH // 128     # 36 l-tiles per core
NSC = S // 512      # 8 s-chunks
NDC = D // 128      # 4 contraction chunks

_CACHE = {}


def _build(repeat=1):
    nc = bacc.Bacc(
        "TRN2", target_bir_lowering=False, debug=False, num_devices=NCORES
    )
    xt = nc.dram_tensor("xt", [D, S], f32r, kind="ExternalInput").ap()
    w1t = nc.dram_tensor("w1t", [D, D], f32r, kind="ExternalInput").ap()
    w2t = nc.dram_tensor("w2t", [128, NDC, LH], f32r, kind="ExternalInput").ap()
    lmask = nc.dram_tensor("lmask", [128, NLT], f32, kind="ExternalInput").ap()
    sel = nc.dram_tensor("sel", [128, 16], f32r, kind="ExternalInput").ap()
    att = nc.dram_tensor("att", [LH, S], f32r, kind="ExternalOutput").ap()
    cs = nc.dram_tensor("cs", [8, 512], f32, kind="ExternalOutput").ap()

    with tile.TileContext(nc) as tc:
        if repeat > 1:
            with tc.For_i(0, repeat, 1):
                _body(tc, att, cs, xt, w1t, w2t, lmask, sel)
        else:
            _body(tc, att, cs, xt, w1t, w2t, lmask, sel)
    nc.compile()
    return nc


def _body(tc, att, cs, xt, w1t, w2t, lmask, sel):
    nc = tc.nc
    with (
        tc.tile_pool(name="p_xt", bufs=NDC) as p_xt,
        tc.tile_pool(name="p_w1t", bufs=NDC) as p_w1t,
        tc.tile_pool(name="p_wt", bufs=NDC) as p_wt,
        tc.tile_pool(name="p_w2t", bufs=3) as p_w2t,
        tc.tile_pool(name="p_exp", bufs=2) as p_exp,
        tc.tile_pool(name="p_small", bufs=2) as p_small,
        tc.tile_pool(name="p_single", bufs=1) as p_single,
        tc.tile_pool(name="p_mm", bufs=3, space="PSUM") as p_mm,
        tc.tile_pool(name="p_cs", bufs=2, space="PSUM") as p_cs,
    ):
        # ---- constants ----
        lmask_sb = p_single.tile([128, NLT], f32, tag="lmask")
        nc.sync.dma_start(out=lmask_sb, in_=lmask)
        sel_sb = p_single.tile([128, 16], f32r, tag="sel")
        nc.sync.dma_start(out=sel_sb, in_=sel)

        # ---- phase 0: wT = tanh(W1 @ xT), resident ----
        xt_sb = []
        w1t_sb = []
        for dc in range(NDC):
            t = p_xt.tile([128, S], f32r, tag="xt")
            nc.sync.dma_start(out=t, in_=xt[dc * 128 : (dc + 1) * 128, :])
            xt_sb.append(t)
            w = p_w1t.tile([128, D], f32r, tag="w1t")
            nc.sync.dma_start(out=w, in_=w1t[dc * 128 : (dc + 1) * 128, :])
            w1t_sb.append(w)

        wt_sb = [p_wt.tile([128, S], f32r, tag="wt", name=f"wt{i}") for i in range(NDC)]
        for ac in range(NDC):
            for sc in range(NSC):
                ps = p_mm.tile([128, 512], f32, tag="mm")
                for dc in range(NDC):
                    nc.tensor.matmul(
                        ps[:],
                        w1t_sb[dc][:, ac * 128 : (ac + 1) * 128],
                        xt_sb[dc][:, sc * 512 : (sc + 1) * 512],
                        start=(dc == 0),
                        stop=(dc == NDC - 1),
                    )
                nc.scalar.activation(
                    out=wt_sb[ac][:, sc * 512 : (sc + 1) * 512],
                    in_=ps[:],
                    func=mybir.ActivationFunctionType.Tanh,
                )

        # ---- phase 1 ----
        cs_ps = [p_cs.tile([4, 512], f32, tag="cs", name=f"cs{i}") for i in range(2)]

        for lt in range(NLT):
            w2t_sb = p_w2t.tile([128, NDC, 128], f32r, tag="w2t")
            nc.sync.dma_start(out=w2t_sb, in_=w2t[:, :, lt * 128 : (lt + 1) * 128])

            exp_sb = p_exp.tile([128, S], f32r, tag="exp")
            sumparts = p_small.tile([128, NSC], f32, tag="sumparts")
            for sc in range(NSC):
                ps = p_mm.tile([128, 512], f32, tag="mm")
                for ac in range(NDC):
                    nc.tensor.matmul(
                        ps[:],
                        w2t_sb[:, ac, :],
                        wt_sb[ac][:, sc * 512 : (sc + 1) * 512],
                        start=(ac == 0),
                        stop=(ac == NDC - 1),
                    )
                nc.scalar.activation(
                    out=exp_sb[:, sc * 512 : (sc + 1) * 512],
                    in_=ps[:],
                    func=mybir.ActivationFunctionType.Exp,
                    accum_out=sumparts[:, sc : sc + 1],
                )

            sumexp = p_small.tile([128, 1], f32, tag="sumexp")
            nc.vector.tensor_reduce(
                out=sumexp[:],
                in_=sumparts[:],
                axis=mybir.AxisListType.X,
                op=mybir.AluOpType.add,
            )
            recip = p_small.tile([128, 1], f32, tag="recip")
            nc.vector.reciprocal(out=recip[:], in_=sumexp[:])
            recipm = p_small.tile([128, 1], f32, tag="recipm")
            nc.vector.tensor_mul(
                out=recipm[:], in0=recip[:], in1=lmask_sb[:, lt : lt + 1]
            )
            # in-place scale: att = exp * (lmask/sumexp)
            nc.vector.tensor_scalar_mul(
                out=exp_sb[:], in0=exp_sb[:].bitcast(f32), scalar1=recipm[:]
            )
            nc.sync.dma_start(
                out=att[lt * 128 : (lt + 1) * 128, :], in_=exp_sb[:]
            )
            # colsum pieces: bank j//... sc 0-3 -> cs_ps[0] rows 0-3, sc 4-7 -> cs_ps[1]
            for sc in range(NSC):
                nc.tensor.matmul(
                    cs_ps[sc // 4][:],
                    sel_sb[:, 4 * (sc % 4) : 4 * (sc % 4) + 4],
                    exp_sb[:, sc * 512 : (sc + 1) * 512],
                    start=(lt == 0 and sc % 4 == 0),
                    stop=(lt == NLT - 1 and sc % 4 == 3),
                    skip_group_check=True,
                )

        cs_sb0 = p_single.tile([4, 512], f32, tag="cs_sb0")
        cs_sb1 = p_single.tile([4, 512], f32, tag="cs_sb1")
        nc.vector.tensor_copy(cs_sb0[:], cs_ps[0][:])
        nc.vector.tensor_copy(cs_sb1[:], cs_ps[1][:])
        nc.sync.dma_start(out=cs[0:4, :], in_=cs_sb0[:])
        nc.sync.dma_start(out=cs[4:8, :], in_=cs_sb1[:])


def _prep_inputs(x, W1, W2):
    """Build per-core input maps (host-side data marshalling)."""
    x = np.ascontiguousarray(np.asarray(x, dtype=np.float32))
    W1 = np.ascontiguousarray(np.asarray(W1, dtype=np.float32))
    W2 = np.ascontiguousarray(np.asarray(W2, dtype=np.float32))

    w1t = np.ascontiguousarray(W1.T)  # [D, D]
    W2pad = np.zeros((2 * LH, D), dtype=np.float32)
    W2pad[:L] = W2

    # selector: block j ([128, 4]) has ones in column j
    sel = np.zeros((128, 16), dtype=np.float32)
    for j in range(4):
        sel[:, 4 * j + j] = 1.0

    in_maps = []
    for c in range(NCORES):
        b, h = divmod(c, 2)
        l0 = h * LH
        # [128, NDC, LH]: w2t[p, dc, l] = W2pad[l0+l, dc*128+p]
        w2t = np.ascontiguousarray(
            W2pad[l0 : l0 + LH].T.reshape(NDC, 128, LH).transpose(1, 0, 2)
        )
        lidx = l0 + np.arange(NLT)[None, :] * 128 + np.arange(128)[:, None]
        lmask = (lidx < L).astype(np.float32)
        in_maps.append(
            {
                "xt": np.ascontiguousarray(x[b].T),
                "w1t": w1t,
                "w2t": w2t,
                "lmask": np.ascontiguousarray(lmask),
                "sel": sel,
            }
        )
    return x, in_maps


def _run(x, W1, W2, trace=False):
    x, in_maps = _prep_inputs(x, W1, W2)
    if "nc" not in _CACHE:
        _CACHE["nc"] = _build()
    nc = _CACHE["nc"]
    res = run_bass_kernel_spmd(nc, in_maps, core_ids=list(range(NCORES)), trace=trace)
    _CACHE["last_results"] = res

    att_full = np.empty((B, L, S), dtype=np.float32)
    out = np.empty((B, D), dtype=np.float32)
    colsum = np.zeros((B, S), dtype=np.float32)
    for c in range(NCORES):
        b, h = divmod(c, 2)
        l0 = h * LH
        rows = min(LH, L - l0)
        att_full[b, l0 : l0 + rows, :] = res.results[c]["att"][:rows]
        colsum[b] += res.results[c]["cs"].reshape(S)
    for b in range(B):
        out[b] = (colsum[b] @ x[b]) / np.float32(R)
    return out, att_full


def kernel(x, W1, W2):
    return _run(x, W1, W2, trace=False)
